# revision 11
# baseline (speedup 1.0000x reference)
"""Trainium2 Bass kernel for nn_Estimator (correlation + 6-conv flow estimator).

Sharding: 8 cores = (batch b in {0,1}) x (H-quarter q in {0..3}).
Each core processes a 74-row slab (64 output rows + 5-row halo each side),
W full (256).  Host slices/pads inputs, gathers outputs.

Per-core device program (single SPMD program, per-core data):
  - correlation cost volume (9x9, mean over 48ch):
      products on GPSIMD (bf16), grouped channel-reduce on VectorE (fp32 out)
      in [w-partition, (dy,dx,c)-free] layout; lrelu(/48 folded into f0t)
      on ScalarE -> bf16; transposed to [q-partition, w] via xbar DMA transpose.
  - conv1 (1x1, 245->160, bf16 matmul) + conv2..conv6 (3x3, fp32 matmuls)
      on TensorE, taps accumulated in PSUM, N=512 (2 rows x 256 cols).
  - lrelu + bias on ScalarE; out-of-image row masking (data-driven, per-core
      mask vector) on GPSIMD so one program serves boundary + interior cores.
"""

import numpy as np
import ml_dtypes

import concourse.bass as bass
import concourse.bacc as bacc
import concourse.mybir as mybir
import concourse.tile as tile
from concourse.ap import AP
from concourse import bass_utils

F32 = mybir.dt.float32
BF16 = mybir.dt.bfloat16

R = 4              # corr radius
ALPHA = 0.1        # leaky relu slope
B, H, W = 2, 256, 256
C0 = 48            # feat channels
HS = 64            # output rows per core
HALO = 5
RS = HS + 2 * HALO       # 74 slab rows
GR = RS + 2 * R          # 82 f1t rows
WP = W + 2 * R           # 264 padded width for f1t / conv rings
NCORE = 8
Q = (2 * R + 1) ** 2     # 81
NG = RS // 2             # 37 row-pair groups
STEPS = NG + 7

D_A = 8            # activation ring depth (+2 margin rows)
D_G = 16           # G ring depth (+1 margin)
D_IN = 8           # input ring depth

# conv chain: (co, ci) ; conv1 separate
CONV = [(160, 245), (128, 160), (112, 128), (96, 112), (64, 96), (4, 64)]

_CACHE = {}


def _mkap(t_ap, offset, dims):
    """Manual AP: dims = [[step,count],...]; offset in elements."""
    return AP(tensor=t_ap.tensor, offset=offset, ap=[list(d) for d in dims])


def _build_nc(reps=1):
    nc = bacc.Bacc("TRN2", target_bir_lowering=False, debug=False)

    # ---------------- DRAM tensors (per-core) ----------------
    d_f01 = nc.dram_tensor("f01", (96, RS, W), BF16, kind="ExternalInput")
    d_lffl = nc.dram_tensor("lffl", (68, RS, W), BF16, kind="ExternalInput")
    d_f0t = nc.dram_tensor("f0t", (RS, W, C0), BF16, kind="ExternalInput")
    d_f1t = nc.dram_tensor("f1t", (GR, WP, C0), BF16, kind="ExternalInput")
    d_mask = nc.dram_tensor("mask", (128, RS), F32, kind="ExternalInput")
    d_bias = nc.dram_tensor("bias", (564, 1), F32, kind="ExternalInput")
    d_w1 = nc.dram_tensor("w1", (245, 160), BF16, kind="ExternalInput")
    d_w2a = nc.dram_tensor("w2a", (128, 9, 128), F32, kind="ExternalInput")
    d_w2b = nc.dram_tensor("w2b", (32, 9, 128), F32, kind="ExternalInput")
    d_w3 = nc.dram_tensor("w3", (128, 9, 112), F32, kind="ExternalInput")
    d_w4 = nc.dram_tensor("w4", (112, 9, 96), F32, kind="ExternalInput")
    d_w5 = nc.dram_tensor("w5", (96, 9, 64), F32, kind="ExternalInput")
    d_w6 = nc.dram_tensor("w6", (64, 9, 4), F32, kind="ExternalInput")
    d_flow = nc.dram_tensor("flow", (4, HS, W), F32, kind="ExternalOutput")
    d_x = nc.dram_tensor("x", (64, HS, W), F32, kind="ExternalOutput")

    with tile.TileContext(nc) as tc:
        for _ in range(reps):
            _emit(nc, tc, d_f01, d_lffl, d_f0t, d_f1t, d_mask, d_bias,
                  [d_w1, d_w2a, d_w2b, d_w3, d_w4, d_w5, d_w6], d_flow, d_x)
    nc.compile()
    return nc


def _emit(nc, tc, d_f01, d_lffl, d_f0t, d_f1t, d_mask, d_bias, d_ws,
          d_flow, d_x, dumps=None):
    d_w1, d_w2a, d_w2b, d_w3, d_w4, d_w5, d_w6 = d_ws
    TAPS = [(dy, dx) for dy in range(3) for dx in range(3)]

    import contextlib
    stack = contextlib.ExitStack()
    const = stack.enter_context(tc.tile_pool(name="const", bufs=1))
    rings = stack.enter_context(tc.tile_pool(name="rings", bufs=1))
    ppool = stack.enter_context(tc.tile_pool(name="prod", bufs=4))
    cpool = stack.enter_context(tc.tile_pool(name="cvt", bufs=3))
    opool = stack.enter_context(tc.tile_pool(name="c6o", bufs=3))
    psum = stack.enter_context(
        tc.tile_pool(name="psum", bufs=1, space="PSUM"))

    # ---------------- constants in SBUF ----------------
    w1_cv = const.tile([81, 160], BF16, tag="w1cv")
    w1_f01 = const.tile([96, 160], BF16, tag="w1f01")
    w1_lffl = const.tile([68, 160], BF16, tag="w1lffl")
    nc.sync.dma_start(w1_cv[:], d_w1[0:81, :])
    nc.sync.dma_start(w1_f01[:], d_w1[81:177, :])
    nc.sync.dma_start(w1_lffl[:], d_w1[177:245, :])
    w2a = const.tile([128, 9, 128], F32, tag="w2a")
    w2b = const.tile([32, 9, 128], F32, tag="w2b")
    w3 = const.tile([128, 9, 112], F32, tag="w3")
    w4 = const.tile([112, 9, 96], F32, tag="w4")
    w5 = const.tile([96, 9, 64], F32, tag="w5")
    w6 = const.tile([64, 9, 4], F32, tag="w6")
    for t, d in ((w2a, d_w2a), (w2b, d_w2b), (w3, d_w3), (w4, d_w4),
                 (w5, d_w5), (w6, d_w6)):
        nc.sync.dma_start(t[:], d[:])
    mask = const.tile([128, RS], F32, tag="mask")
    nc.sync.dma_start(mask[:], d_mask[:])
    # biases: slices of d_bias; rows 0:128,128:160 -> b1 tiles; etc.
    boffs = [(0, 128), (128, 32), (160, 128), (288, 112), (400, 96),
             (496, 64), (560, 4)]
    btiles = []
    for i, (o, n) in enumerate(boffs):
        bt = const.tile([n, 1], F32, tag=f"b{i}")
        nc.sync.dma_start(bt[:], d_bias[o:o + n, :])
        btiles.append(bt)
    b1a, b1b, b2, b3, b4, b5, b6 = btiles

    # ---------------- ring buffers ----------------
    # activation rings [co, D_A+2, WP] fp32  (margin slots 8,9 dup 0,1)
    a1a = rings.tile([128, D_A + 2, WP], F32, tag="a1a")
    a1b = rings.tile([32, D_A + 2, WP], F32, tag="a1b")
    a2 = rings.tile([128, D_A + 2, WP], F32, tag="a2")
    a3 = rings.tile([112, D_A + 2, WP], F32, tag="a3")
    a4 = rings.tile([96, D_A + 2, WP], F32, tag="a4")
    a5 = rings.tile([64, D_A + 2, WP], F32, tag="a5")
    for t in (a1a, a1b, a2, a3, a4, a5):
        nc.gpsimd.memset(t[:], 0.0)
    # input rings
    f01r = rings.tile([96, D_IN, W], BF16, tag="f01r")
    lfflr = rings.tile([68, D_IN, W], BF16, tag="lfflr")
    cvr = rings.tile([128, D_IN, W], BF16, tag="cvr")      # post-transpose cv
    # (rows 0:81 hold the cost volume; 81:128 are transpose spill, unread)
    f0t_r = [rings.tile([128, D_IN, C0], BF16, tag=f"f0t{wb}",
                        name=f"f0t{wb}") for wb in range(2)]
    g_r = [rings.tile([128, D_G + 1, 9 * C0], BF16, tag=f"g{wb}",
                      name=f"g{wb}") for wb in range(2)]

    # ---------------- DMA helpers ----------------
    def load_g_row(grow):
        """im2col G row: [w:128][dx:9][c:48] overlapping windows."""
        if grow >= GR:
            return
        s = grow % D_G
        for wb in range(2):
            src = _mkap(d_f1t.ap(), (grow * WP + wb * 128) * C0,
                        [[C0, 128], [C0, 9], [1, C0]])
            dst = g_r[wb][:, s, :].rearrange("p (a b) -> p a b", a=9)
            nc.sync.dma_start(dst, src)
            if s == 0:
                dstm = g_r[wb][:, D_G, :].rearrange("p (a b) -> p a b", a=9)
                nc.sync.dma_start(dstm, src)

    def load_f0t_row(r):
        if r >= RS:
            return
        s = r % D_IN
        for wb in range(2):
            src = _mkap(d_f0t.ap(), (r * W + wb * 128) * C0,
                        [[C0, 128], [1, C0]])
            nc.sync.dma_start(f0t_r[wb][:, s, :], src)

    def load_in_rows(r):
        if r >= RS:
            return
        s = r % D_IN
        nc.sync.dma_start(f01r[:, s:s + 2, :], d_f01[:, r:r + 2, :])
        nc.sync.dma_start(lfflr[:, s:s + 2, :], d_lffl[:, r:r + 2, :])

    # ---------------- correlation for row pair (2g, 2g+1) ----------------
    def corr(g):
        r0 = 2 * g
        si = r0 % D_IN
        for wb in range(2):
            cvt = cpool.tile([128, 2, Q], F32, tag=f"cvt{wb}")
            # partition stride & row stride from the natural AP:
            nat = f0t_r[wb][:, si:si + 2, :]  # [128, 2, 48]
            pstep = nat.ap[0][0]
            rstep = nat.ap[1][0]
            f0b = _mkap(nat, nat.offset,
                        [[pstep, 128], [rstep, 2], [0, 9], [1, C0]])
            for dy in range(9):
                b2 = (r0 + dy) % D_G
                gn = g_r[wb][:, b2:b2 + 2, :]  # [128, 2, 432]
                gap = _mkap(gn, gn.offset,
                            [[gn.ap[0][0], 128], [gn.ap[1][0], 2],
                             [C0, 9], [1, C0]])
                prod = ppool.tile([128, 2, 9, C0], BF16, tag=f"p{wb}")
                nc.gpsimd.tensor_tensor(prod[:], f0b, gap,
                                        op=mybir.AluOpType.mult)
                nc.vector.tensor_reduce(
                    cvt[:, :, dy * 9:(dy + 1) * 9], prod[:],
                    axis=mybir.AxisListType.X, op=mybir.AluOpType.add)
            # lrelu (scale 1/48 pre-folded into f0t) -> bf16
            # padded to 128 free cols for the xbar transpose (tile 16x128)
            cvb = cpool.tile([128, 2, 128], BF16, tag=f"cvb{wb}")
            nc.scalar.activation(cvb[:, :, 0:Q], cvt[:],
                                 mybir.ActivationFunctionType.Prelu,
                                 alpha=ALPHA)
            nc.scalar.mul(cvb[:, :, Q:128], cvb[:, :, Q:128], 0.0)
            for i in range(2):
                so = (r0 + i) % D_IN
                nc.sync.dma_start_transpose(
                    cvr[:, so, wb * 128:(wb + 1) * 128], cvb[:, i, :])

    # ---------------- convs ----------------
    EDGE = {0, 1, 2, NG - 3, NG - 2, NG - 1}

    def mask_rows(ring, co, p):
        s = (2 * p) % D_A
        m = mask[0:co, 2 * p:2 * p + 2]
        mb = _mkap(m, m.offset, [[m.ap[0][0], co], [1, 2], [0, W]])
        io = ring[0:co, s:s + 2, R:R + W]
        nc.gpsimd.tensor_tensor(io, io, mb, op=mybir.AluOpType.mult)

    def dup_margin(ring, p):
        if (2 * p) % D_A == 0:
            nc.sync.dma_start(ring[:, D_A:D_A + 2, :], ring[:, 0:2, :])

    def conv1(p):
        si = (2 * p) % D_IN
        sa = (2 * p) % D_A
        chunks = [(w1_cv, cvr[0:81, si:si + 2, :]),
                  (w1_f01, f01r[:, si:si + 2, :]),
                  (w1_lffl, lfflr[:, si:si + 2, :])]
        for mt, (mo, mn, ring, bt) in enumerate(
                [(0, 128, a1a, b1a), (128, 32, a1b, b1b)]):
            ps = psum.tile([mn, 2, W], F32, tag=f"ps1{mt}")
            for ci, (wt, rhs) in enumerate(chunks):
                nc.tensor.matmul(ps[:], wt[:, mo:mo + mn], rhs,
                                 start=(ci == 0), stop=(ci == 2))
            nc.scalar.activation(ring[0:mn, sa:sa + 2, R:R + W], ps[:],
                                 mybir.ActivationFunctionType.Prelu,
                                 bias=bt[:], alpha=ALPHA)
            if p in EDGE:
                mask_rows(ring, mn, p)
            dup_margin(ring, p)

    def conv3x3(p, wts, in_rings, co, out_ring, bt, final=False):
        """wts: list of (wtile, ci) matching in_rings K-chunks."""
        ps = psum.tile([co, 2, W], F32, tag=f"ps{id(out_ring) % 97}"
                       if False else f"ps_{co}_{len(wts)}")
        n_mm = 9 * len(wts)
        k = 0
        for dy, dx in TAPS:
            base = (2 * p - 1 + dy) % D_A
            for (wt, ci), ring in zip(wts, in_rings):
                rhs = ring[0:ci, base:base + 2, R + dx - 1:R + dx - 1 + W]
                nc.tensor.matmul(ps[:], wt[:, 3 * dy + dx, :], rhs,
                                 start=(k == 0), stop=(k == n_mm - 1))
                k += 1
        if final:
            ot = opool.tile([4, 2, W], F32, tag="c6o")
            nc.scalar.activation(ot[:], ps[:],
                                 mybir.ActivationFunctionType.Identity,
                                 bias=bt[:])
            return ot
        sa = (2 * p) % D_A
        nc.scalar.activation(out_ring[0:co, sa:sa + 2, R:R + W], ps[:],
                             mybir.ActivationFunctionType.Prelu,
                             bias=bt[:], alpha=ALPHA)
        if p in EDGE:
            mask_rows(out_ring, co, p)
        dup_margin(out_ring, p)
        return None

    def emit_out(p, src_rows_ap_fn, dst):
        """Write rows of pair p intersected with [5, 69) to dst."""
        r0 = 2 * p
        if r0 + 1 < HALO or r0 >= HS + HALO:
            return
        lo = max(r0, HALO)
        hi = min(r0 + 2, HS + HALO)
        src = src_rows_ap_fn(lo - r0, hi - r0)
        nc.sync.dma_start(dst[:, lo - HALO:hi - HALO, :], src)

    # ---------------- preloads ----------------
    for grow in range(12):
        load_g_row(grow)
    load_f0t_row(0)
    load_f0t_row(1)
    load_f0t_row(2)
    load_f0t_row(3)

    def dump_rows(key, ring, co, p):
        if dumps is None or key not in dumps:
            return
        s = (2 * p) % (D_A if ring is not cvr else D_IN)
        nc.sync.dma_start(dumps[key][0:co, 2 * p:2 * p + 2, :],
                          ring[0:co, s:s + 2, R:R + W]
                          if ring is not cvr else ring[0:co, s:s + 2, :])

    # ---------------- main loop ----------------
    for g in range(STEPS):
        if g < NG:
            load_g_row(2 * g + 12)
            load_g_row(2 * g + 13)
            load_f0t_row(2 * g + 4)
            load_f0t_row(2 * g + 5)
            load_in_rows(2 * g)
            corr(g)
            dump_rows("cvd", cvr, 128, g)
        p = g - 2
        if 0 <= p < NG:
            conv1(p)
            dump_rows("a1d", a1a, 128, p)
            dump_rows("a1bd", a1b, 32, p)
        p = g - 3
        if 0 <= p < NG:
            conv3x3(p, [(w2a, 128), (w2b, 32)], [a1a, a1b], 128, a2, b2)
            dump_rows("a2d", a2, 128, p)
        p = g - 4
        if 0 <= p < NG:
            conv3x3(p, [(w3, 128)], [a2], 112, a3, b3)
            dump_rows("a3d", a3, 112, p)
        p = g - 5
        if 0 <= p < NG:
            conv3x3(p, [(w4, 112)], [a3], 96, a4, b4)
            dump_rows("a4d", a4, 96, p)
        p = g - 6
        if 0 <= p < NG:
            conv3x3(p, [(w5, 96)], [a4], 64, a5, b5)
            dump_rows("a5d", a5, 64, p)
            sa = (2 * p) % D_A
            emit_out(p, lambda i0, i1, _sa=sa: a5[:, _sa + i0:_sa + i1,
                                                  R:R + W], d_x)
        p = g - 7
        if 0 <= p < NG:
            ot = conv3x3(p, [(w6, 64)], [a5], 4, None, b6, final=True)
            emit_out(p, lambda i0, i1, _ot=ot: _ot[:, i0:i1, :], d_flow)

    stack.close()


# ======================= host side =======================

def _row_slab(arr, h0, lo_off, n_rows):
    """arr (C, H, W) -> (C, n_rows, W) rows [h0+lo_off, ...), zero padded."""
    C = arr.shape[0]
    out = np.zeros((C, n_rows, arr.shape[2]), np.float32)
    lo = h0 + lo_off
    s0, s1 = max(lo, 0), min(lo + n_rows, H)
    if s1 > s0:
        out[:, s0 - lo:s1 - lo, :] = arr[:, s0:s1, :]
    return out


def _prep_core(inputs, b, q):
    h0 = q * HS
    f0 = np.asarray(inputs["feat0"][b], np.float32)
    f1 = np.asarray(inputs["feat1"][b], np.float32)
    lf = np.asarray(inputs["last_feat"][b], np.float32)
    fl = np.asarray(inputs["last_flow"][b], np.float32)

    f0s = _row_slab(f0, h0, -HALO, RS)
    f1s = _row_slab(f1, h0, -HALO, RS)
    lfs = _row_slab(lf, h0, -HALO, RS)
    fls = _row_slab(fl, h0, -HALO, RS)

    bf = ml_dtypes.bfloat16
    f01 = np.concatenate([f0s, f1s], 0).astype(bf)
    lffl = np.concatenate([lfs, fls], 0).astype(bf)
    f0t = (f0s.transpose(1, 2, 0) / 48.0).astype(bf)
    f1g = _row_slab(f1, h0, -HALO - R, GR)
    f1t = np.zeros((GR, WP, C0), np.float32)
    f1t[:, R:R + W, :] = f1g.transpose(1, 2, 0)
    f1t = f1t.astype(bf)

    m = np.zeros(RS, np.float32)
    for r in range(RS):
        hr = h0 - HALO + r
        m[r] = 1.0 if 0 <= hr < H else 0.0
    maskrep = np.tile(m[None, :], (128, 1)).astype(np.float32)
    return {"f01": f01, "lffl": lffl, "f0t": f0t, "f1t": f1t,
            "mask": maskrep}


def _prep_weights(inputs):
    bf = ml_dtypes.bfloat16
    w1 = np.asarray(inputs["w1"], np.float32)[:, :, 0, 0]   # (160,245)
    out = {"w1": np.ascontiguousarray(w1.T).astype(bf)}
    wt = {}
    for n in range(2, 7):
        w = np.asarray(inputs[f"w{n}"], np.float32)          # (co,ci,3,3)
        wt[n] = np.ascontiguousarray(w.transpose(1, 2, 3, 0).reshape(
            w.shape[1], 9, w.shape[0])).astype(np.float32)
    out["w2a"] = wt[2][:128]
    out["w2b"] = np.ascontiguousarray(wt[2][128:])
    out["w3"], out["w4"], out["w5"], out["w6"] = wt[3], wt[4], wt[5], wt[6]
    bias = np.zeros((564, 1), np.float32)
    o = 0
    for n, co in zip(range(1, 7), (160, 128, 112, 96, 64, 4)):
        bias[o:o + co, 0] = np.asarray(inputs[f"b{n}"], np.float32)
        o += co
    out["bias"] = bias
    return out


def get_nc():
    if "nc" not in _CACHE:
        _CACHE["nc"] = _build_nc()
    return _CACHE["nc"]


def kernel(**inputs):
    nc = get_nc()
    wmap = _prep_weights(inputs)
    in_maps = []
    for k in range(NCORE):
        b, q = k // 4, k % 4
        m = _prep_core(inputs, b, q)
        m.update(wmap)
        in_maps.append(m)
    res = bass_utils.run_bass_kernel_spmd(nc, in_maps,
                                          core_ids=list(range(NCORE)))
    flow = np.zeros((B, 4, H, W), np.float32)
    x = np.zeros((B, 64, H, W), np.float32)
    for k in range(NCORE):
        b, q = k // 4, k % 4
        flow[b, :, q * HS:(q + 1) * HS, :] = res.results[k]["flow"]
        x[b, :, q * HS:(q + 1) * HS, :] = res.results[k]["x"]
    return flow, x
